# revision 3
# baseline (speedup 1.0000x reference)
"""DVH loss kernel for Trainium2 (8 NeuronCores, SPMD data-parallel over voxels).

Math (see reference): with D_BINS = arange(0,85), masks = y_true[:,1:23],
dose_t = y_true[:,0], dose_p = relu(y_pred[:,0]):
    dvh_t[s,d] = sum_n masks[n,s] * sigmoid(dose_t[n] - d)
    dvh_p[s,d] = sum_n masks[n,s] * sigmoid(dose_p[n] - d)
    tot[s]     = sum_n masks[n,s] + eps
    loss = sum_s ||(dvh_t[s]-dvh_p[s])/tot[s]||_2 / (85*22)

Device strategy (per core, 249984 voxels = 1953 chunks of 128):
  - stream y_true / y_pred blocks of 21 chunks, partition-major contiguous DMA
  - split doses into bf16 hi+lo (exactness), transpose via PE so a tiny
    K=5 bf16 matmul builds the sigmoid argument tiles (dose - d) in PSUM
  - one batched ACT sigmoid per 3 chunks: PSUM f32 -> SBUF bf16
  - PE accumulates masks^T @ [sig_t | sig_p | 1] into a [22,171] PSUM tile
Host: sums the 8 partial [22,171] outputs (+ a 128-row tail computed in
numpy) and does the tiny final norm/loss reduction.
"""

import numpy as np
import ml_dtypes

ND = 85
NS = 23
EPS = float(np.finfo(np.float32).eps)
NSND = float(ND) * (NS - 1)

N_CORES = 8
P = 128
BLK_CHUNKS = 21            # chunks per block
GRP_CHUNKS = 3             # chunks per sigmoid batch (one PSUM bank of args)
BLOCKS = 93                # blocks per core
CHUNKS = BLK_CHUNKS * BLOCKS        # 1953 chunks/core
ROWS_PER_CORE = CHUNKS * P          # 249984
DEV_ROWS = ROWS_PER_CORE * N_CORES  # 1999872 (tail rows handled on host)

NF = 2 * ND + 1            # 171: [sig_t(85) | sig_p(85) | ones(1)]

_BUILD_CACHE = {}


def _host_consts():
    # Selector constants for the arg-build matmul:
    #   args[:, jj*170+d] = sum_r doseT[r, :] * cst[r, (g), jj*170+d]
    # doseT rows: 0:21 hi_t, 21:42 hi_p, 42:63 lo_t, 63:84 lo_p, 84:105 ones.
    # For chunk j = g*3+jj, t-args need hi_t[j] + lo_t[j] - d, p likewise.
    n_grp = BLK_CHUNKS // GRP_CHUNKS
    FW = GRP_CHUNKS * 2 * ND  # 510
    cst = np.zeros((5 * BLK_CHUNKS, n_grp * FW), np.float32)
    dbin = np.arange(ND, dtype=np.float32)
    for g in range(n_grp):
        for jj in range(GRP_CHUNKS):
            j = g * GRP_CHUNKS + jj
            c0 = g * FW + jj * 2 * ND
            cst[j, c0:c0 + ND] = 1.0                     # hi_t
            cst[42 + j, c0:c0 + ND] = 1.0                # lo_t
            cst[21 + j, c0 + ND:c0 + 2 * ND] = 1.0       # hi_p
            cst[63 + j, c0 + ND:c0 + 2 * ND] = 1.0       # lo_p
            cst[84 + j, c0:c0 + ND] = -dbin              # ones -> -d
            cst[84 + j, c0 + ND:c0 + 2 * ND] = -dbin
    cst16 = cst.astype(ml_dtypes.bfloat16)
    idn = np.eye(P, dtype=ml_dtypes.bfloat16)
    return cst16, idn


def _build_nc(blocks=BLOCKS):
    """Build the per-core Bass program (identical for all 8 cores)."""
    from contextlib import ExitStack

    import concourse.bacc as bacc
    import concourse.bass as bass
    import concourse.tile as tile
    from concourse import mybir

    f32 = mybir.dt.float32
    bf16 = mybir.dt.bfloat16
    rows = blocks * BLK_CHUNKS * P

    nc = bacc.Bacc("TRN2", target_bir_lowering=False, debug=False)

    yt = nc.dram_tensor("yt", [rows, NS], f32, kind="ExternalInput")
    yp = nc.dram_tensor("yp", [rows, NS], f32, kind="ExternalInput")
    n_grp = BLK_CHUNKS // GRP_CHUNKS
    FW = GRP_CHUNKS * 2 * ND
    cst = nc.dram_tensor("cst", [5 * BLK_CHUNKS, n_grp * FW], bf16, kind="ExternalInput")
    idn = nc.dram_tensor("idn", [P, P], bf16, kind="ExternalInput")
    out = nc.dram_tensor("out", [NS - 1, NF], f32, kind="ExternalOutput")

    # block b, partition p, sub-row j, channel c:  row = (b*128 + p)*21 + j
    ytr = yt[:].rearrange("(b p j) c -> b p (j c)", p=P, j=BLK_CHUNKS)
    ypr = yp[:].rearrange("(b p j) c -> b p (j c)", p=P, j=BLK_CHUNKS)

    W = BLK_CHUNKS              # 21
    FB = W * NS                 # 483 floats per partition per block

    with ExitStack() as ctx:
        tc = ctx.enter_context(tile.TileContext(nc))
        singles = ctx.enter_context(tc.tile_pool(name="singles", bufs=1))
        ld = ctx.enter_context(tc.tile_pool(name="ld", bufs=3))
        work = ctx.enter_context(tc.tile_pool(name="work", bufs=2))
        sigp = ctx.enter_context(tc.tile_pool(name="sigp", bufs=4))
        pargs = ctx.enter_context(tc.tile_pool(name="pargs", bufs=2, space="PSUM"))
        ptr = ctx.enter_context(tc.tile_pool(name="ptr", bufs=2, space="PSUM"))
        pmain = ctx.enter_context(tc.tile_pool(name="pmain", bufs=1, space="PSUM"))

        cst_sb = singles.tile([5 * BLK_CHUNKS, n_grp * FW], bf16)
        nc.sync.dma_start(out=cst_sb[:], in_=cst[:])
        idn_sb = singles.tile([P, P], bf16)
        nc.sync.dma_start(out=idn_sb[:], in_=idn[:])

        mains = pmain.tile([NS - 1, NF], f32)

        for b in range(blocks):
            yt_t = ld.tile([P, FB], f32, tag="yt")
            nc.sync.dma_start(out=yt_t[:], in_=ytr[b])
            yp_t = ld.tile([P, FB], f32, tag="yp")
            nc.sync.dma_start(out=yp_t[:], in_=ypr[b])

            yt3 = yt_t[:].rearrange("p (j c) -> p j c", c=NS)
            yp3 = yp_t[:].rearrange("p (j c) -> p j c", c=NS)

            # bf16 copy of y_true (for mask weights)
            yt16 = work.tile([P, FB], bf16, tag="yt16")
            nc.vector.tensor_copy(out=yt16[:], in_=yt_t[:])
            yt16_3 = yt16[:].rearrange("p (j c) -> p j c", c=NS)

            # doses: [t (21) | relu(p) (21)]
            dose2 = work.tile([P, 2 * W], f32, tag="dose2")
            nc.vector.tensor_copy(out=dose2[:, 0:W], in_=yt3[:, :, 0])
            nc.vector.tensor_scalar_max(dose2[:, W:2 * W], yp3[:, :, 0], 0.0)

            # hi/lo bf16 split + ones block: dose16 = [hi(42) | lo(42) | ones(21)]
            dose16 = work.tile([P, 4 * W + W], bf16, tag="dose16")
            nc.vector.tensor_copy(out=dose16[:, 0:2 * W], in_=dose2[:])
            hi32 = work.tile([P, 2 * W], f32, tag="hi32")
            nc.vector.tensor_copy(out=hi32[:], in_=dose16[:, 0:2 * W])
            lo32 = work.tile([P, 2 * W], f32, tag="lo32")
            nc.vector.tensor_tensor(out=lo32[:], in0=dose2[:], in1=hi32[:],
                                    op=mybir.AluOpType.subtract)
            nc.vector.tensor_copy(out=dose16[:, 2 * W:4 * W], in_=lo32[:])
            nc.vector.memset(dose16[:, 4 * W:5 * W], 1.0)

            # transpose -> doseT [105, 128] bf16 (rows: hi_t | hi_p | lo_t | lo_p | ones)
            tr_ps = ptr.tile([5 * W, P], bf16, tag="trps")
            nc.tensor.transpose(tr_ps[:], dose16[:], idn_sb[:])
            doseT = work.tile([5 * W, P], bf16, tag="doseT")
            nc.vector.tensor_copy(out=doseT[:], in_=tr_ps[:])

            for g in range(n_grp):
                args = pargs.tile([P, FW], f32, tag="args")
                nc.tensor.matmul(
                    args[:], doseT[:], cst_sb[:, g * FW:(g + 1) * FW],
                    start=True, stop=True)

                sig = sigp.tile([P, GRP_CHUNKS, NF], bf16, tag="sig")
                nc.vector.memset(sig[:, :, 2 * ND:NF], 1.0)
                nc.scalar.activation(
                    out=sig[:, :, 0:2 * ND],
                    in_=args[:].rearrange("p (j d) -> p j d", d=2 * ND),
                    func=mybir.ActivationFunctionType.Sigmoid)

                for jj in range(GRP_CHUNKS):
                    j = g * GRP_CHUNKS + jj
                    first = (b == 0 and g == 0 and jj == 0)
                    last = (b == blocks - 1 and g == n_grp - 1 and jj == GRP_CHUNKS - 1)
                    nc.tensor.matmul(
                        mains[:], yt16_3[:, j, 1:NS], sig[:, jj, :],
                        start=first, stop=last)

        out_sb = singles.tile([NS - 1, NF], f32)
        nc.vector.tensor_copy(out=out_sb[:], in_=mains[:])
        nc.sync.dma_start(out=out[:], in_=out_sb[:])

    nc.compile()
    return nc


def _get_nc(blocks=BLOCKS):
    if blocks not in _BUILD_CACHE:
        _BUILD_CACHE[blocks] = _build_nc(blocks)
    return _BUILD_CACHE[blocks]


def _host_partial(y_true, y_pred):
    """Numpy fp32 dvh partials for a (small) row range: returns [22, 171] acc."""
    d = np.arange(ND, dtype=np.float32)
    m = y_true[:, 1:NS].astype(np.float32)
    m = (m != 0).astype(np.float32)
    xt = y_true[:, 0:1].astype(np.float32)
    xp = np.maximum(y_pred[:, 0:1].astype(np.float32), 0.0)
    st = 1.0 / (1.0 + np.exp(-(xt - d), dtype=np.float64))
    sp = 1.0 / (1.0 + np.exp(-(xp - d), dtype=np.float64))
    acc = np.zeros((NS - 1, NF), np.float64)
    acc[:, 0:ND] = m.T @ st
    acc[:, ND:2 * ND] = m.T @ sp
    acc[:, 2 * ND] = m.sum(axis=0)
    return acc


def kernel(y_true: np.ndarray, y_pred: np.ndarray) -> np.ndarray:
    from concourse.bass_utils import run_bass_kernel_spmd

    y_true = np.asarray(y_true)
    y_pred = np.asarray(y_pred)
    n = y_true.shape[0]
    assert y_true.shape == (n, NS) and y_pred.shape == (n, NS)
    assert n >= DEV_ROWS, f"kernel hardcoded for N=2000000, got {n}"

    nc = _get_nc()
    cst16, idn = _host_consts()

    yt8 = y_true[:DEV_ROWS].reshape(N_CORES, ROWS_PER_CORE, NS)
    yp8 = y_pred[:DEV_ROWS].reshape(N_CORES, ROWS_PER_CORE, NS)
    in_maps = [
        {
            "yt": np.ascontiguousarray(yt8[i], dtype=np.float32),
            "yp": np.ascontiguousarray(yp8[i], dtype=np.float32),
            "cst": cst16,
            "idn": idn,
        }
        for i in range(N_CORES)
    ]
    res = run_bass_kernel_spmd(nc, in_maps, core_ids=list(range(N_CORES)))

    acc = np.zeros((NS - 1, NF), np.float64)
    for r in res.results:
        acc += r["out"].astype(np.float64)
    if n > DEV_ROWS:
        acc += _host_partial(y_true[DEV_ROWS:], y_pred[DEV_ROWS:])

    dvh_t = acc[:, 0:ND]
    dvh_p = acc[:, ND:2 * ND]
    tot = acc[:, 2 * ND:2 * ND + 1] + EPS
    diff = (dvh_t - dvh_p) / tot
    per_s = np.sqrt((diff * diff).sum(axis=1))
    loss = per_s.sum() / NSND
    return np.asarray(loss, dtype=np.float32)
